# revision 16
# baseline (speedup 1.0000x reference)
"""Trainium2 Bass kernel for nn_CustomizedLinear (masked pathway linear).

out[b, p*768+e] = sum_d x[b,d] * (weight*mask.T)[p,d] * G[d,e] + bias[p]
with B=64, P=256, D=2000, E=768.

Sharding: tensor-parallel over the pathway dim P — 32 pathways per core on
8 cores; x and gene_embedding replicated.

Per-core compute: for each pathway p, scale x columns by wm[p] (DVE
broadcast multiply) and matmul with G. The TensorE matmul costs N cycles
per instruction regardless of K/M, so we pack 2 pathways x 64 batch rows
into the stationary operand (M=128) and stream G in N=384 chunks,
accumulating 16 k-tiles of 125 into PSUM. All matmul operands are
float32r (fp32 with 11-bit mantissa, 1 cycle/row vs 4 for fp32; rel err
~1.5e-4 at this depth). Input/output DMAs are split across both HWDGE
rings (SP + Activation) so G streaming does not starve the PE.
"""
import sys

sys.path.insert(0, "/opt/trn_rl_repo")

import numpy as np
from contextlib import ExitStack

import concourse.bacc as bacc
import concourse.tile as tile
import concourse.mybir as mybir
from concourse.bass_utils import run_bass_kernel_spmd

F32 = mybir.dt.float32
F32R = mybir.dt.float32r

N_CORES = 8
B = 64          # batch
D = 2000        # genes (contraction)
E = 768         # embedding
P_TOT = 256     # pathways
P_CORE = P_TOT // N_CORES        # 32 pathways per core
KT = 16                          # k-tiles
KP = D // KT                     # 125 rows per k-tile
NCH = 2                          # N chunks per pair
NC_N = E // NCH                  # 384


def _build_program(repeat=1, groups=8, split_rings=True, psum_bufs=8,
                   strip_bufs=6, g_chunks=(1, 1, 2, 4, 4, 4), g_rings=(0, 1)):
    gp = P_CORE // groups        # pathways per group (must be even)
    npair = gp // 2
    nc = bacc.Bacc()
    # x/w/m arrive host-permuted and packed into one [KP, KT*(B+2*P_CORE)]
    # tensor: [x (KT*B) | w (KT*P_CORE) | m (KT*P_CORE)] so one DMA loads all
    XWM_W = KT * (B + 2 * P_CORE)
    xwm_d = nc.declare_dram_parameter("xwm", [KP, XWM_W], F32, isOutput=False)
    g_d = nc.declare_dram_parameter("g", [D, E], F32, isOutput=False)
    bias_d = nc.declare_dram_parameter("bias", [2 * B, P_CORE // 2], F32,
                                       isOutput=False)
    out_d = nc.declare_dram_parameter("out", [B, P_CORE * E], F32, isOutput=True)

    def ring(i):
        if not split_rings:
            return nc.sync
        return nc.sync if i % 2 == 0 else nc.scalar

    with tile.TileContext(nc) as tc, ExitStack() as ctx:
        const = ctx.enter_context(tc.tile_pool(name="const", bufs=1))
        stage = ctx.enter_context(tc.tile_pool(name="stage", bufs=3))
        strips = ctx.enter_context(tc.tile_pool(name="strips", bufs=strip_bufs))
        outs = ctx.enter_context(tc.tile_pool(name="outs", bufs=4))
        psum = ctx.enter_context(
            tc.tile_pool(name="psum", bufs=psum_bufs, space="PSUM"))

        bias_t = const.tile([2 * B, P_CORE // 2], F32)
        nc.scalar.dma_start(out=bias_t[:], in_=bias_d[:])

        # x/w/m first (small, gate the strip pipeline): one packed DMA
        xwm = const.tile([KP, XWM_W], F32)
        nc.scalar.dma_start(out=xwm[:], in_=xwm_d[:])
        x_all = xwm[:, :KT * B]
        w_all = xwm[:, KT * B:KT * (B + P_CORE)]
        m_all = xwm[:, KT * (B + P_CORE):]
        wm_all = const.tile([KP, KT * P_CORE], F32)
        nc.vector.tensor_mul(wm_all[:], w_all, m_all)
        x_t = [x_all[:, B * k:B * (k + 1)] for k in range(KT)]
        wm_t = [wm_all[:, P_CORE * k:P_CORE * (k + 1)] for k in range(KT)]

        # G stream: uneven chunks so the first cast starts early; ring
        # placement per g_rings; casts to f32r on the idle gpsimd engine
        g_view = g_d[:].rearrange("(k d) e -> d k e", k=KT)
        g_r = []
        k0 = 0
        for c, w in enumerate(g_chunks):
            gs = const.tile([KP, w * E], F32, name=f"gs{c}")
            dst = gs[:].rearrange("d (k e) -> d k e", k=w)
            eng = nc.sync if g_rings[c % len(g_rings)] == 0 else nc.scalar
            eng.dma_start(out=dst, in_=g_view[:, k0:k0 + w, :])
            for j in range(w):
                gr = const.tile([KP, E], F32R, tag=f"g{k0 + j}",
                                name=f"g{k0 + j}")
                nc.gpsimd.tensor_copy(gr[:], gs[:, E * j:E * (j + 1)])
                g_r.append(gr)
            k0 += w
        assert k0 == KT

        out_p = out_d[:].rearrange("b (p e) -> p b e", p=P_CORE)  # [32, 64, 768]

        if repeat > 1:
            loop_cm = tc.For_i(0, repeat, 1,
                               hint_engines=(mybir.EngineType.PE,))
            loop_cm.__enter__()

        odma = [0]
        for g in range(groups):
            ps = [psum.tile([2 * B, NC_N], F32, tag="ps", name=f"ps{g}_{i}")
                  for i in range(npair * NCH)]
            for k in range(KT):
                st = strips.tile([KP, gp * B], F32R, tag="strip",
                                 name=f"st{g}_{k}")
                st3 = st[:].rearrange("d (p b) -> d p b", p=gp)
                x_bc = x_t[k][:].unsqueeze(1).broadcast_to([KP, gp, B])
                w_bc = (wm_t[k][:, gp * g:gp * (g + 1)]
                        .unsqueeze(2).broadcast_to([KP, gp, B]))
                nc.vector.tensor_mul(st3, x_bc, w_bc)
                for pair in range(npair):
                    lhsT = st[:, 2 * B * pair:2 * B * (pair + 1)]
                    for n in range(NCH):
                        nc.tensor.matmul(
                            ps[NCH * pair + n][:],
                            lhsT,
                            g_r[k][:, NC_N * n:NC_N * (n + 1)],
                            start=(k == 0),
                            stop=(k == KT - 1),
                        )
            for pair in range(npair):
                pg = npair * g + pair          # global pair index 0..15
                o = outs.tile([2 * B, E], F32, tag="o", name=f"o{g}_{pair}")
                for n in range(NCH):
                    nc.scalar.activation(
                        o[:, NC_N * n:NC_N * (n + 1)], ps[NCH * pair + n][:],
                        mybir.ActivationFunctionType.Identity,
                        bias=bias_t[:, pg:pg + 1],
                    )
                p0 = 2 * pg
                dst = out_p[p0:p0 + 2, :, :]
                ring(odma[0]).dma_start(out=dst, in_=o[:])
                odma[0] += 1

        if repeat > 1:
            loop_cm.__exit__(None, None, None)

    nc.finalize()
    return nc


_NC_CACHE = None


def _get_program():
    global _NC_CACHE
    if _NC_CACHE is None:
        _NC_CACHE = _build_program()
    return _NC_CACHE


def _make_in_maps(x, weight, bias, mask, gene_embedding):
    def kperm(a):  # (D, W) -> (KP, KT*W) with [d, k*W+w] = a[k*KP+d, w]
        w = a.shape[1]
        return np.ascontiguousarray(
            a.reshape(KT, KP, w).transpose(1, 0, 2).reshape(KP, KT * w))

    xT = kperm(x.T)                                      # (125, 16*64)
    in_maps = []
    for c in range(N_CORES):
        sl = slice(P_CORE * c, P_CORE * (c + 1))
        wT_c = kperm(weight[sl].T)                       # (125, 16*32)
        mk_c = kperm(mask[:, sl])                        # (125, 16*32)
        xwm = np.ascontiguousarray(np.concatenate([xT, wT_c, mk_c], axis=1))
        b_c = bias[sl]
        # (128, 16): col i = [bias[2i]]*64 ++ [bias[2i+1]]*64
        bias_sb = np.ascontiguousarray(
            np.repeat(b_c.reshape(P_CORE // 2, 2), B, axis=1).T)
        in_maps.append({"xwm": xwm, "g": gene_embedding, "bias": bias_sb})
    return in_maps


def kernel(x, weight, bias, mask, gene_embedding, _want_results=False, **_):
    x = np.ascontiguousarray(x, dtype=np.float32)
    weight = np.ascontiguousarray(weight, dtype=np.float32)
    bias = np.ascontiguousarray(bias, dtype=np.float32)
    mask = np.ascontiguousarray(mask, dtype=np.float32)
    g = np.ascontiguousarray(gene_embedding, dtype=np.float32)

    in_maps = _make_in_maps(x, weight, bias, mask, g)
    nc = _get_program()
    res = run_bass_kernel_spmd(nc, in_maps, list(range(N_CORES)))
    out = np.concatenate([r["out"] for r in res.results], axis=1)
    if _want_results:
        return out, res
    return out


# revision 17
# speedup vs baseline: 1.0197x; 1.0197x over previous
"""Trainium2 Bass kernel for nn_CustomizedLinear (masked pathway linear).

out[b, p*768+e] = sum_d x[b,d] * (weight*mask.T)[p,d] * G[d,e] + bias[p]
with B=64, P=256, D=2000, E=768.

Sharding: tensor-parallel over the pathway dim P — 32 pathways per core on
8 cores; x and gene_embedding replicated.

Per-core compute: for each pathway p, scale x columns by wm[p] (DVE
broadcast multiply) and matmul with G. The TensorE matmul costs N cycles
per instruction regardless of K/M, so we pack 2 pathways x 64 batch rows
into the stationary operand (M=128) and stream G in N=384 chunks,
accumulating 16 k-tiles of 125 into PSUM. All matmul operands are
float32r (fp32 with 11-bit mantissa, 1 cycle/row vs 4 for fp32; rel err
~1.5e-4 at this depth). Input/output DMAs are split across both HWDGE
rings (SP + Activation) so G streaming does not starve the PE.
"""
import sys

sys.path.insert(0, "/opt/trn_rl_repo")

import numpy as np
from contextlib import ExitStack

import concourse.bacc as bacc
import concourse.tile as tile
import concourse.mybir as mybir
from concourse.bass_utils import run_bass_kernel_spmd

F32 = mybir.dt.float32
F32R = mybir.dt.float32r

N_CORES = 8
B = 64          # batch
D = 2000        # genes (contraction)
E = 768         # embedding
P_TOT = 256     # pathways
P_CORE = P_TOT // N_CORES        # 32 pathways per core
KT = 16                          # k-tiles
KP = D // KT                     # 125 rows per k-tile
NCH = 2                          # N chunks per pair
NC_N = E // NCH                  # 384


def _build_program(repeat=1, groups=8, split_rings=True, psum_bufs=8,
                   strip_bufs=6, g_chunks=(1,) * KT, g_rings=(0, 1)):
    gp = P_CORE // groups        # pathways per group (must be even)
    npair = gp // 2
    nc = bacc.Bacc()
    # x/w/m arrive host-permuted and packed into one [KP, KT*(B+2*P_CORE)]
    # tensor: [x (KT*B) | w (KT*P_CORE) | m (KT*P_CORE)] so one DMA loads all
    XWM_W = KT * (B + 2 * P_CORE)
    xwm_d = nc.declare_dram_parameter("xwm", [KP, XWM_W], F32, isOutput=False)
    g_d = nc.declare_dram_parameter("g", [D, E], F32, isOutput=False)
    bias_d = nc.declare_dram_parameter("bias", [2 * B, P_CORE // 2], F32,
                                       isOutput=False)
    out_d = nc.declare_dram_parameter("out", [B, P_CORE * E], F32, isOutput=True)

    def ring(i):
        if not split_rings:
            return nc.sync
        return nc.sync if i % 2 == 0 else nc.scalar

    with tile.TileContext(nc) as tc, ExitStack() as ctx:
        const = ctx.enter_context(tc.tile_pool(name="const", bufs=1))
        stage = ctx.enter_context(tc.tile_pool(name="stage", bufs=3))
        strips = ctx.enter_context(tc.tile_pool(name="strips", bufs=strip_bufs))
        outs = ctx.enter_context(tc.tile_pool(name="outs", bufs=4))
        psum = ctx.enter_context(
            tc.tile_pool(name="psum", bufs=psum_bufs, space="PSUM"))

        bias_t = const.tile([2 * B, P_CORE // 2], F32)
        nc.scalar.dma_start(out=bias_t[:], in_=bias_d[:])

        # x/w/m first (small, gate the strip pipeline): one packed DMA
        xwm = const.tile([KP, XWM_W], F32)
        nc.scalar.dma_start(out=xwm[:], in_=xwm_d[:])
        x_all = xwm[:, :KT * B]
        w_all = xwm[:, KT * B:KT * (B + P_CORE)]
        m_all = xwm[:, KT * (B + P_CORE):]
        wm_all = const.tile([KP, KT * P_CORE], F32)
        nc.vector.tensor_mul(wm_all[:], w_all, m_all)
        x_t = [x_all[:, B * k:B * (k + 1)] for k in range(KT)]
        wm_t = [wm_all[:, P_CORE * k:P_CORE * (k + 1)] for k in range(KT)]

        # G stream: uneven chunks so the first cast starts early; ring
        # placement per g_rings; casts to f32r on the idle gpsimd engine
        g_view = g_d[:].rearrange("(k d) e -> d k e", k=KT)
        g_r = []
        k0 = 0
        for c, w in enumerate(g_chunks):
            gs = const.tile([KP, w * E], F32, name=f"gs{c}")
            dst = gs[:].rearrange("d (k e) -> d k e", k=w)
            eng = nc.sync if g_rings[c % len(g_rings)] == 0 else nc.scalar
            eng.dma_start(out=dst, in_=g_view[:, k0:k0 + w, :])
            for j in range(w):
                gr = const.tile([KP, E], F32R, tag=f"g{k0 + j}",
                                name=f"g{k0 + j}")
                nc.gpsimd.tensor_copy(gr[:], gs[:, E * j:E * (j + 1)])
                g_r.append(gr)
            k0 += w
        assert k0 == KT

        out_p = out_d[:].rearrange("b (p e) -> p b e", p=P_CORE)  # [32, 64, 768]

        if repeat > 1:
            loop_cm = tc.For_i(0, repeat, 1,
                               hint_engines=(mybir.EngineType.PE,))
            loop_cm.__enter__()

        odma = [0]
        for g in range(groups):
            ps = [psum.tile([2 * B, NC_N], F32, tag="ps", name=f"ps{g}_{i}")
                  for i in range(npair * NCH)]
            for k in range(KT):
                st = strips.tile([KP, gp * B], F32R, tag="strip",
                                 name=f"st{g}_{k}")
                st3 = st[:].rearrange("d (p b) -> d p b", p=gp)
                x_bc = x_t[k][:].unsqueeze(1).broadcast_to([KP, gp, B])
                w_bc = (wm_t[k][:, gp * g:gp * (g + 1)]
                        .unsqueeze(2).broadcast_to([KP, gp, B]))
                nc.vector.tensor_mul(st3, x_bc, w_bc)
                for pair in range(npair):
                    lhsT = st[:, 2 * B * pair:2 * B * (pair + 1)]
                    for n in range(NCH):
                        nc.tensor.matmul(
                            ps[NCH * pair + n][:],
                            lhsT,
                            g_r[k][:, NC_N * n:NC_N * (n + 1)],
                            start=(k == 0),
                            stop=(k == KT - 1),
                        )
            for pair in range(npair):
                pg = npair * g + pair          # global pair index 0..15
                o = outs.tile([2 * B, E], F32, tag="o", name=f"o{g}_{pair}")
                for n in range(NCH):
                    nc.scalar.activation(
                        o[:, NC_N * n:NC_N * (n + 1)], ps[NCH * pair + n][:],
                        mybir.ActivationFunctionType.Identity,
                        bias=bias_t[:, pg:pg + 1],
                    )
                p0 = 2 * pg
                dst = out_p[p0:p0 + 2, :, :]
                ring(odma[0]).dma_start(out=dst, in_=o[:])
                odma[0] += 1

        if repeat > 1:
            loop_cm.__exit__(None, None, None)

    nc.finalize()
    return nc


_NC_CACHE = None


def _get_program():
    global _NC_CACHE
    if _NC_CACHE is None:
        _NC_CACHE = _build_program()
    return _NC_CACHE


def _make_in_maps(x, weight, bias, mask, gene_embedding):
    def kperm(a):  # (D, W) -> (KP, KT*W) with [d, k*W+w] = a[k*KP+d, w]
        w = a.shape[1]
        return np.ascontiguousarray(
            a.reshape(KT, KP, w).transpose(1, 0, 2).reshape(KP, KT * w))

    xT = kperm(x.T)                                      # (125, 16*64)
    in_maps = []
    for c in range(N_CORES):
        sl = slice(P_CORE * c, P_CORE * (c + 1))
        wT_c = kperm(weight[sl].T)                       # (125, 16*32)
        mk_c = kperm(mask[:, sl])                        # (125, 16*32)
        xwm = np.ascontiguousarray(np.concatenate([xT, wT_c, mk_c], axis=1))
        b_c = bias[sl]
        # (128, 16): col i = [bias[2i]]*64 ++ [bias[2i+1]]*64
        bias_sb = np.ascontiguousarray(
            np.repeat(b_c.reshape(P_CORE // 2, 2), B, axis=1).T)
        in_maps.append({"xwm": xwm, "g": gene_embedding, "bias": bias_sb})
    return in_maps


def kernel(x, weight, bias, mask, gene_embedding, _want_results=False, **_):
    x = np.ascontiguousarray(x, dtype=np.float32)
    weight = np.ascontiguousarray(weight, dtype=np.float32)
    bias = np.ascontiguousarray(bias, dtype=np.float32)
    mask = np.ascontiguousarray(mask, dtype=np.float32)
    g = np.ascontiguousarray(gene_embedding, dtype=np.float32)

    in_maps = _make_in_maps(x, weight, bias, mask, g)
    nc = _get_program()
    res = run_bass_kernel_spmd(nc, in_maps, list(range(N_CORES)))
    out = np.concatenate([r["out"] for r in res.results], axis=1)
    if _want_results:
        return out, res
    return out


# revision 20
# speedup vs baseline: 1.0322x; 1.0123x over previous
"""Trainium2 Bass kernel for nn_CustomizedLinear (masked pathway linear).

out[b, p*768+e] = sum_d x[b,d] * (weight*mask.T)[p,d] * G[d,e] + bias[p]
with B=64, P=256, D=2000, E=768.

Sharding: tensor-parallel over the pathway dim P — 32 pathways per core on
8 cores; x and gene_embedding replicated.

Per-core compute: for each pathway p, scale x columns by wm[p] (DVE
broadcast multiply) and matmul with G. The TensorE matmul costs N cycles
per instruction regardless of K/M, so we pack 2 pathways x 64 batch rows
into the stationary operand (M=128) and stream G in N=384 chunks,
accumulating 16 k-tiles of 125 into PSUM. All matmul operands are
float32r (fp32 with 11-bit mantissa, 1 cycle/row vs 4 for fp32; rel err
~1.5e-4 at this depth). Input/output DMAs are split across both HWDGE
rings (SP + Activation) so G streaming does not starve the PE.
"""
import sys

sys.path.insert(0, "/opt/trn_rl_repo")

import numpy as np
from contextlib import ExitStack

import concourse.bacc as bacc
import concourse.tile as tile
import concourse.mybir as mybir
from concourse.bass_utils import run_bass_kernel_spmd

F32 = mybir.dt.float32
F32R = mybir.dt.float32r

N_CORES = 8
B = 64          # batch
D = 2000        # genes (contraction)
E = 768         # embedding
P_TOT = 256     # pathways
P_CORE = P_TOT // N_CORES        # 32 pathways per core
KT = 16                          # k-tiles
KP = D // KT                     # 125 rows per k-tile
NCH = 2                          # N chunks per pair
NC_N = E // NCH                  # 384


def _build_program(repeat=1, groups=8, split_rings=True, psum_bufs=8,
                   strip_bufs=6, g_chunks=(1,) * KT, g_rings=(0, 1)):
    gp = P_CORE // groups        # pathways per group (must be even)
    npair = gp // 2
    nc = bacc.Bacc()
    # x/w/m arrive host-permuted k-major: per k-tile a contiguous block
    # [x_k (B) | w_k (P_CORE) | m_k (P_CORE)]; a small head DMA (k=0,1)
    # lets the strip pipeline start before the bulk load finishes
    BLK = B + 2 * P_CORE
    XWM_W = KT * BLK
    HEAD = 1
    xwm_d = nc.declare_dram_parameter("xwm", [KP, XWM_W], F32, isOutput=False)
    g_d = nc.declare_dram_parameter("g", [D, E], F32, isOutput=False)
    bias_d = nc.declare_dram_parameter("bias", [2 * B, P_CORE // 2], F32,
                                       isOutput=False)
    out_d = nc.declare_dram_parameter("out", [B, P_CORE * E], F32, isOutput=True)

    def ring(i):
        if not split_rings:
            return nc.sync
        return nc.sync if i % 2 == 0 else nc.scalar

    with tile.TileContext(nc) as tc, ExitStack() as ctx:
        const = ctx.enter_context(tc.tile_pool(name="const", bufs=1))
        stage = ctx.enter_context(tc.tile_pool(name="stage", bufs=3))
        strips = ctx.enter_context(tc.tile_pool(name="strips", bufs=strip_bufs))
        outs = ctx.enter_context(tc.tile_pool(name="outs", bufs=4))
        psum = ctx.enter_context(
            tc.tile_pool(name="psum", bufs=psum_bufs, space="PSUM"))

        # x/w/m: head (k<HEAD) now; tail spliced into the G stream below
        xwm_h = const.tile([KP, HEAD * BLK], F32)
        nc.scalar.dma_start(out=xwm_h[:], in_=xwm_d[:, :HEAD * BLK])
        bias_t = const.tile([2 * B, P_CORE // 2], F32)
        nc.scalar.dma_start(out=bias_t[:], in_=bias_d[:])
        xwm_t = const.tile([KP, (KT - HEAD) * BLK], F32)

        def blk(k):
            t = xwm_h if k < HEAD else xwm_t
            o = BLK * (k if k < HEAD else k - HEAD)
            return t[:, o:o + BLK]

        # G stream: uneven chunks so the first cast starts early; ring
        # placement per g_rings; casts to f32r on the idle gpsimd engine
        g_view = g_d[:].rearrange("(k d) e -> d k e", k=KT)
        g_r = []
        k0 = 0
        for c, w in enumerate(g_chunks):
            if c == 3:  # xwm tail after G has a head start
                nc.scalar.dma_start(out=xwm_t[:], in_=xwm_d[:, HEAD * BLK:])
            gs = const.tile([KP, w * E], F32, name=f"gs{c}")
            dst = gs[:].rearrange("d (k e) -> d k e", k=w)
            eng = nc.sync if g_rings[c % len(g_rings)] == 0 else nc.scalar
            eng.dma_start(out=dst, in_=g_view[:, k0:k0 + w, :])
            for j in range(w):
                gr = const.tile([KP, E], F32R, tag=f"g{k0 + j}",
                                name=f"g{k0 + j}")
                cast_eng = nc.vector if k0 + j < 2 else nc.gpsimd
                cast_eng.tensor_copy(gr[:], gs[:, E * j:E * (j + 1)])
                g_r.append(gr)
            k0 += w
        assert k0 == KT

        x_t, wm_t = [], []
        for k in range(KT):
            b = blk(k)
            x_t.append(b[:, :B])
            wm = const.tile([KP, P_CORE], F32, tag=f"wm{k}", name=f"wm{k}")
            nc.vector.tensor_mul(wm[:], b[:, B:B + P_CORE], b[:, B + P_CORE:])
            wm_t.append(wm)


        out_p = out_d[:].rearrange("b (p e) -> p b e", p=P_CORE)  # [32, 64, 768]

        if repeat > 1:
            loop_cm = tc.For_i(0, repeat, 1,
                               hint_engines=(mybir.EngineType.PE,))
            loop_cm.__enter__()

        odma = [0]
        for g in range(groups):
            ps = [psum.tile([2 * B, NC_N], F32, tag="ps", name=f"ps{g}_{i}")
                  for i in range(npair * NCH)]
            for k in range(KT):
                st = strips.tile([KP, gp * B], F32R, tag="strip",
                                 name=f"st{g}_{k}")
                st3 = st[:].rearrange("d (p b) -> d p b", p=gp)
                x_bc = x_t[k][:].unsqueeze(1).broadcast_to([KP, gp, B])
                w_bc = (wm_t[k][:, gp * g:gp * (g + 1)]
                        .unsqueeze(2).broadcast_to([KP, gp, B]))
                nc.vector.tensor_mul(st3, x_bc, w_bc)
                for pair in range(npair):
                    lhsT = st[:, 2 * B * pair:2 * B * (pair + 1)]
                    for n in range(NCH):
                        nc.tensor.matmul(
                            ps[NCH * pair + n][:],
                            lhsT,
                            g_r[k][:, NC_N * n:NC_N * (n + 1)],
                            start=(k == 0),
                            stop=(k == KT - 1),
                        )
            for pair in range(npair):
                pg = npair * g + pair          # global pair index 0..15
                last = (g == groups - 1 and pair == npair - 1)
                o = outs.tile([2 * B, E], F32, tag="o", name=f"o{g}_{pair}")
                p0 = 2 * pg
                for n in range(NCH):
                    nc.scalar.activation(
                        o[:, NC_N * n:NC_N * (n + 1)], ps[NCH * pair + n][:],
                        mybir.ActivationFunctionType.Identity,
                        bias=bias_t[:, pg:pg + 1],
                    )
                    if last:
                        dst = out_p[p0:p0 + 2, :, NC_N * n:NC_N * (n + 1)]
                        ring(odma[0]).dma_start(
                            out=dst, in_=o[:, NC_N * n:NC_N * (n + 1)])
                        odma[0] += 1
                if not last:
                    dst = out_p[p0:p0 + 2, :, :]
                    ring(odma[0]).dma_start(out=dst, in_=o[:])
                    odma[0] += 1

        if repeat > 1:
            loop_cm.__exit__(None, None, None)

    nc.finalize()
    return nc


_NC_CACHE = None


def _get_program():
    global _NC_CACHE
    if _NC_CACHE is None:
        _NC_CACHE = _build_program()
    return _NC_CACHE


def _make_in_maps(x, weight, bias, mask, gene_embedding):
    def kperm(a):  # (D, W) -> (KP, KT*W) with [d, k*W+w] = a[k*KP+d, w]
        w = a.shape[1]
        return np.ascontiguousarray(
            a.reshape(KT, KP, w).transpose(1, 0, 2).reshape(KP, KT * w))

    xT = x.T.reshape(KT, KP, B)                          # (16, 125, 64)
    in_maps = []
    for c in range(N_CORES):
        sl = slice(P_CORE * c, P_CORE * (c + 1))
        wT_c = weight[sl].T.reshape(KT, KP, P_CORE)
        mk_c = mask[:, sl].reshape(KT, KP, P_CORE)
        # k-major blocks [x_k | w_k | m_k] -> (125, 16*(64+32+32))
        xwm = np.ascontiguousarray(
            np.concatenate([xT, wT_c, mk_c], axis=2)
            .transpose(1, 0, 2).reshape(KP, -1))
        b_c = bias[sl]
        # (128, 16): col i = [bias[2i]]*64 ++ [bias[2i+1]]*64
        bias_sb = np.ascontiguousarray(
            np.repeat(b_c.reshape(P_CORE // 2, 2), B, axis=1).T)
        in_maps.append({"xwm": xwm, "g": gene_embedding, "bias": bias_sb})
    return in_maps


def kernel(x, weight, bias, mask, gene_embedding, _want_results=False, **_):
    x = np.ascontiguousarray(x, dtype=np.float32)
    weight = np.ascontiguousarray(weight, dtype=np.float32)
    bias = np.ascontiguousarray(bias, dtype=np.float32)
    mask = np.ascontiguousarray(mask, dtype=np.float32)
    g = np.ascontiguousarray(gene_embedding, dtype=np.float32)

    in_maps = _make_in_maps(x, weight, bias, mask, g)
    nc = _get_program()
    res = run_bass_kernel_spmd(nc, in_maps, list(range(N_CORES)))
    out = np.concatenate([r["out"] for r in res.results], axis=1)
    if _want_results:
        return out, res
    return out


# revision 22
# speedup vs baseline: 1.0359x; 1.0036x over previous
"""Trainium2 Bass kernel for nn_CustomizedLinear (masked pathway linear).

out[b, p*768+e] = sum_d x[b,d] * (weight*mask.T)[p,d] * G[d,e] + bias[p]
with B=64, P=256, D=2000, E=768.

Sharding: tensor-parallel over the pathway dim P — 32 pathways per core on
8 cores; x and gene_embedding replicated.

Per-core compute: for each pathway p, scale x columns by wm[p] (DVE
broadcast multiply) and matmul with G. The TensorE matmul costs N cycles
per instruction regardless of K/M, so we pack 2 pathways x 64 batch rows
into the stationary operand (M=128) and stream G in N=384 chunks,
accumulating 16 k-tiles of 125 into PSUM. All matmul operands are
float32r (fp32 with 11-bit mantissa, 1 cycle/row vs 4 for fp32; rel err
~1.5e-4 at this depth). Input/output DMAs are split across both HWDGE
rings (SP + Activation) so G streaming does not starve the PE.
"""
import sys

sys.path.insert(0, "/opt/trn_rl_repo")

import numpy as np
from contextlib import ExitStack

import concourse.bacc as bacc
import concourse.tile as tile
import concourse.mybir as mybir
from concourse.bass_utils import run_bass_kernel_spmd

F32 = mybir.dt.float32
F32R = mybir.dt.float32r

N_CORES = 8
B = 64          # batch
D = 2000        # genes (contraction)
E = 768         # embedding
P_TOT = 256     # pathways
P_CORE = P_TOT // N_CORES        # 32 pathways per core
KT = 16                          # k-tiles
KP = D // KT                     # 125 rows per k-tile
NCH = 2                          # N chunks per pair
NC_N = E // NCH                  # 384


def _build_program(repeat=1, groups=8, split_rings=True, psum_bufs=8,
                   strip_bufs=6, g_chunks=(1,) * KT, g_rings=(0, 0, 1)):
    gp = P_CORE // groups        # pathways per group (must be even)
    npair = gp // 2
    nc = bacc.Bacc()
    # x/w/m arrive host-permuted k-major: per k-tile a contiguous block
    # [x_k (B) | w_k (P_CORE) | m_k (P_CORE)]; a small head DMA (k=0,1)
    # lets the strip pipeline start before the bulk load finishes
    BLK = B + 2 * P_CORE
    XWM_W = KT * BLK
    HEAD = 4
    xwm_d = nc.declare_dram_parameter("xwm", [KP, XWM_W], F32, isOutput=False)
    g_d = nc.declare_dram_parameter("g", [D, E], F32, isOutput=False)
    bias_d = nc.declare_dram_parameter("bias", [2 * B, P_CORE // 2], F32,
                                       isOutput=False)
    out_d = nc.declare_dram_parameter("out", [B, P_CORE * E], F32, isOutput=True)

    def ring(i):
        if not split_rings:
            return nc.sync
        return nc.sync if i % 2 == 0 else nc.scalar

    with tile.TileContext(nc) as tc, ExitStack() as ctx:
        const = ctx.enter_context(tc.tile_pool(name="const", bufs=1))
        stage = ctx.enter_context(tc.tile_pool(name="stage", bufs=3))
        strips = ctx.enter_context(tc.tile_pool(name="strips", bufs=strip_bufs))
        outs = ctx.enter_context(tc.tile_pool(name="outs", bufs=4))
        psum = ctx.enter_context(
            tc.tile_pool(name="psum", bufs=psum_bufs, space="PSUM"))

        # x/w/m: head (k<HEAD) now; tail spliced into the G stream below
        xwm_h = const.tile([KP, HEAD * BLK], F32)
        nc.scalar.dma_start(out=xwm_h[:], in_=xwm_d[:, :HEAD * BLK])
        bias_t = const.tile([2 * B, P_CORE // 2], F32)
        nc.scalar.dma_start(out=bias_t[:], in_=bias_d[:])
        MID = 10
        xwm_t1 = const.tile([KP, (MID - HEAD) * BLK], F32)
        xwm_t2 = const.tile([KP, (KT - MID) * BLK], F32)

        def blk(k):
            if k < HEAD:
                return xwm_h[:, BLK * k:BLK * (k + 1)]
            if k < MID:
                o = BLK * (k - HEAD)
                return xwm_t1[:, o:o + BLK]
            o = BLK * (k - MID)
            return xwm_t2[:, o:o + BLK]

        # G stream: uneven chunks so the first cast starts early; ring
        # placement per g_rings; casts to f32r on the idle gpsimd engine
        g_view = g_d[:].rearrange("(k d) e -> d k e", k=KT)
        g_r = []
        k0 = 0
        for c, w in enumerate(g_chunks):
            if c == 2:  # xwm tail pt1 after G has a head start
                nc.scalar.dma_start(out=xwm_t1[:],
                                    in_=xwm_d[:, HEAD * BLK:MID * BLK])
            if c == 4:
                nc.scalar.dma_start(out=xwm_t2[:], in_=xwm_d[:, MID * BLK:])
            gs = const.tile([KP, w * E], F32, name=f"gs{c}")
            dst = gs[:].rearrange("d (k e) -> d k e", k=w)
            eng = nc.sync if g_rings[c % len(g_rings)] == 0 else nc.scalar
            eng.dma_start(out=dst, in_=g_view[:, k0:k0 + w, :])
            for j in range(w):
                gr = const.tile([KP, E], F32R, tag=f"g{k0 + j}",
                                name=f"g{k0 + j}")
                cast_eng = nc.vector if k0 + j < 2 else nc.gpsimd
                cast_eng.tensor_copy(gr[:], gs[:, E * j:E * (j + 1)])
                g_r.append(gr)
            k0 += w
        assert k0 == KT

        x_t, wm_t = [], []
        for k in range(KT):
            b = blk(k)
            x_t.append(b[:, :B])
            wm = const.tile([KP, P_CORE], F32, tag=f"wm{k}", name=f"wm{k}")
            nc.vector.tensor_mul(wm[:], b[:, B:B + P_CORE], b[:, B + P_CORE:])
            wm_t.append(wm)


        out_p = out_d[:].rearrange("b (p e) -> p b e", p=P_CORE)  # [32, 64, 768]

        if repeat > 1:
            loop_cm = tc.For_i(0, repeat, 1,
                               hint_engines=(mybir.EngineType.PE,))
            loop_cm.__enter__()

        odma = [0]
        for g in range(groups):
            ps = [psum.tile([2 * B, NC_N], F32, tag="ps", name=f"ps{g}_{i}")
                  for i in range(npair * NCH)]
            for k in range(KT):
                st = strips.tile([KP, gp * B], F32R, tag="strip",
                                 name=f"st{g}_{k}")
                st3 = st[:].rearrange("d (p b) -> d p b", p=gp)
                x_bc = x_t[k][:].unsqueeze(1).broadcast_to([KP, gp, B])
                w_bc = (wm_t[k][:, gp * g:gp * (g + 1)]
                        .unsqueeze(2).broadcast_to([KP, gp, B]))
                nc.vector.tensor_mul(st3, x_bc, w_bc)
                for pair in range(npair):
                    lhsT = st[:, 2 * B * pair:2 * B * (pair + 1)]
                    for n in range(NCH):
                        nc.tensor.matmul(
                            ps[NCH * pair + n][:],
                            lhsT,
                            g_r[k][:, NC_N * n:NC_N * (n + 1)],
                            start=(k == 0),
                            stop=(k == KT - 1),
                        )
            for pair in range(npair):
                pg = npair * g + pair          # global pair index 0..15
                last = (g == groups - 1 and pair == npair - 1)
                o = outs.tile([2 * B, E], F32, tag="o", name=f"o{g}_{pair}")
                p0 = 2 * pg
                for n in range(NCH):
                    nc.scalar.activation(
                        o[:, NC_N * n:NC_N * (n + 1)], ps[NCH * pair + n][:],
                        mybir.ActivationFunctionType.Identity,
                        bias=bias_t[:, pg:pg + 1],
                    )
                    if last:
                        dst = out_p[p0:p0 + 2, :, NC_N * n:NC_N * (n + 1)]
                        ring(odma[0]).dma_start(
                            out=dst, in_=o[:, NC_N * n:NC_N * (n + 1)])
                        odma[0] += 1
                if not last:
                    dst = out_p[p0:p0 + 2, :, :]
                    ring(odma[0]).dma_start(out=dst, in_=o[:])
                    odma[0] += 1

        if repeat > 1:
            loop_cm.__exit__(None, None, None)

    nc.finalize()
    return nc


_NC_CACHE = None


def _get_program():
    global _NC_CACHE
    if _NC_CACHE is None:
        _NC_CACHE = _build_program()
    return _NC_CACHE


def _make_in_maps(x, weight, bias, mask, gene_embedding):
    def kperm(a):  # (D, W) -> (KP, KT*W) with [d, k*W+w] = a[k*KP+d, w]
        w = a.shape[1]
        return np.ascontiguousarray(
            a.reshape(KT, KP, w).transpose(1, 0, 2).reshape(KP, KT * w))

    xT = x.T.reshape(KT, KP, B)                          # (16, 125, 64)
    in_maps = []
    for c in range(N_CORES):
        sl = slice(P_CORE * c, P_CORE * (c + 1))
        wT_c = weight[sl].T.reshape(KT, KP, P_CORE)
        mk_c = mask[:, sl].reshape(KT, KP, P_CORE)
        # k-major blocks [x_k | w_k | m_k] -> (125, 16*(64+32+32))
        xwm = np.ascontiguousarray(
            np.concatenate([xT, wT_c, mk_c], axis=2)
            .transpose(1, 0, 2).reshape(KP, -1))
        b_c = bias[sl]
        # (128, 16): col i = [bias[2i]]*64 ++ [bias[2i+1]]*64
        bias_sb = np.ascontiguousarray(
            np.repeat(b_c.reshape(P_CORE // 2, 2), B, axis=1).T)
        in_maps.append({"xwm": xwm, "g": gene_embedding, "bias": bias_sb})
    return in_maps


def kernel(x, weight, bias, mask, gene_embedding, _want_results=False, **_):
    x = np.ascontiguousarray(x, dtype=np.float32)
    weight = np.ascontiguousarray(weight, dtype=np.float32)
    bias = np.ascontiguousarray(bias, dtype=np.float32)
    mask = np.ascontiguousarray(mask, dtype=np.float32)
    g = np.ascontiguousarray(gene_embedding, dtype=np.float32)

    in_maps = _make_in_maps(x, weight, bias, mask, g)
    nc = _get_program()
    res = run_bass_kernel_spmd(nc, in_maps, list(range(N_CORES)))
    out = np.concatenate([r["out"] for r in res.results], axis=1)
    if _want_results:
        return out, res
    return out


# revision 23
# speedup vs baseline: 1.0968x; 1.0588x over previous
"""Trainium2 Bass kernel for nn_CustomizedLinear (masked pathway linear).

out[b, p*768+e] = sum_d x[b,d] * (weight*mask.T)[p,d] * G[d,e] + bias[p]
with B=64, P=256, D=2000, E=768.

Sharding: tensor-parallel over the pathway dim P — 32 pathways per core on
8 cores; x and gene_embedding replicated.

Per-core compute: for each pathway p, scale x columns by wm[p] (DVE
broadcast multiply) and matmul with G. The TensorE matmul costs N cycles
per instruction regardless of K/M, so we pack 2 pathways x 64 batch rows
into the stationary operand (M=128) and stream G in N=384 chunks,
accumulating 16 k-tiles of 125 into PSUM. All matmul operands are
float32r (fp32 with 11-bit mantissa, 1 cycle/row vs 4 for fp32; rel err
~1.5e-4 at this depth). Input/output DMAs are split across both HWDGE
rings (SP + Activation) so G streaming does not starve the PE.
"""
import sys

sys.path.insert(0, "/opt/trn_rl_repo")

import numpy as np
from contextlib import ExitStack

import concourse.bacc as bacc
import concourse.tile as tile
import concourse.mybir as mybir
from concourse.bass_utils import run_bass_kernel_spmd

F32 = mybir.dt.float32
F32R = mybir.dt.float32r

N_CORES = 8
B = 64          # batch
D = 2000        # genes (contraction)
E = 768         # embedding
P_TOT = 256     # pathways
P_CORE = P_TOT // N_CORES        # 32 pathways per core
KT = 16                          # k-tiles
KP = D // KT                     # 125 rows per k-tile
NCH = 2                          # N chunks per pair
NC_N = E // NCH                  # 384


def _build_program(repeat=1, group_sizes=(8, 4, 4, 4, 4, 4, 2, 2),
                   split_rings=True, psum_bufs=8,
                   strip_bufs=6, g_chunks=(1,) * KT, g_rings=(0, 0, 1)):
    assert sum(group_sizes) == P_CORE
    nc = bacc.Bacc()
    # x/w/m arrive host-permuted k-major: per k-tile a contiguous block
    # [x_k (B) | w_k (P_CORE) | m_k (P_CORE)]; a small head DMA (k=0,1)
    # lets the strip pipeline start before the bulk load finishes
    BLK = B + 2 * P_CORE
    XWM_W = KT * BLK
    HEAD = 4
    xwm_d = nc.declare_dram_parameter("xwm", [KP, XWM_W], F32, isOutput=False)
    g_d = nc.declare_dram_parameter("g", [D, E], F32, isOutput=False)
    bias_d = nc.declare_dram_parameter("bias", [2 * B, P_CORE // 2], F32,
                                       isOutput=False)
    out_d = nc.declare_dram_parameter("out", [B, P_CORE * E], F32, isOutput=True)

    def ring(i):
        if not split_rings:
            return nc.sync
        return nc.sync if i % 2 == 0 else nc.scalar

    with tile.TileContext(nc) as tc, ExitStack() as ctx:
        const = ctx.enter_context(tc.tile_pool(name="const", bufs=1))
        stage = ctx.enter_context(tc.tile_pool(name="stage", bufs=3))
        strips = ctx.enter_context(tc.tile_pool(name="strips", bufs=strip_bufs))
        outs = ctx.enter_context(tc.tile_pool(name="outs", bufs=4))
        psum = ctx.enter_context(
            tc.tile_pool(name="psum", bufs=psum_bufs, space="PSUM"))

        # x/w/m: head (k<HEAD) now; tail spliced into the G stream below
        xwm_h = const.tile([KP, HEAD * BLK], F32)
        nc.scalar.dma_start(out=xwm_h[:], in_=xwm_d[:, :HEAD * BLK])
        bias_t = const.tile([2 * B, P_CORE // 2], F32)
        nc.scalar.dma_start(out=bias_t[:], in_=bias_d[:])
        MID = 10
        xwm_t1 = const.tile([KP, (MID - HEAD) * BLK], F32)
        xwm_t2 = const.tile([KP, (KT - MID) * BLK], F32)

        def blk(k):
            if k < HEAD:
                return xwm_h[:, BLK * k:BLK * (k + 1)]
            if k < MID:
                o = BLK * (k - HEAD)
                return xwm_t1[:, o:o + BLK]
            o = BLK * (k - MID)
            return xwm_t2[:, o:o + BLK]

        # G stream: uneven chunks so the first cast starts early; ring
        # placement per g_rings; casts to f32r on the idle gpsimd engine
        g_view = g_d[:].rearrange("(k d) e -> d k e", k=KT)
        g_r = []
        k0 = 0
        for c, w in enumerate(g_chunks):
            if c == 2:  # xwm tail pt1 after G has a head start
                nc.scalar.dma_start(out=xwm_t1[:],
                                    in_=xwm_d[:, HEAD * BLK:MID * BLK])
            if c == 4:
                nc.scalar.dma_start(out=xwm_t2[:], in_=xwm_d[:, MID * BLK:])
            gs = const.tile([KP, w * E], F32, name=f"gs{c}")
            dst = gs[:].rearrange("d (k e) -> d k e", k=w)
            eng = nc.sync if g_rings[c % len(g_rings)] == 0 else nc.scalar
            eng.dma_start(out=dst, in_=g_view[:, k0:k0 + w, :])
            for j in range(w):
                gr = const.tile([KP, E], F32R, tag=f"g{k0 + j}",
                                name=f"g{k0 + j}")
                cast_eng = nc.vector if k0 + j < 2 else nc.gpsimd
                cast_eng.tensor_copy(gr[:], gs[:, E * j:E * (j + 1)])
                g_r.append(gr)
            k0 += w
        assert k0 == KT

        x_t, wm_t = [], []
        for k in range(KT):
            b = blk(k)
            x_t.append(b[:, :B])
            wm = const.tile([KP, P_CORE], F32, tag=f"wm{k}", name=f"wm{k}")
            nc.vector.tensor_mul(wm[:], b[:, B:B + P_CORE], b[:, B + P_CORE:])
            wm_t.append(wm)


        out_p = out_d[:].rearrange("b (p e) -> p b e", p=P_CORE)  # [32, 64, 768]

        if repeat > 1:
            loop_cm = tc.For_i(0, repeat, 1,
                               hint_engines=(mybir.EngineType.PE,))
            loop_cm.__enter__()

        odma = [0]
        p_start = 0
        for g, gp in enumerate(group_sizes):
            npair = gp // 2
            ps = [psum.tile([2 * B, NC_N], F32, tag="ps", name=f"ps{g}_{i}")
                  for i in range(npair * NCH)]
            for k in range(KT):
                st = strips.tile([KP, gp * B], F32R, tag=f"strip{gp}",
                                 name=f"st{g}_{k}")
                st3 = st[:].rearrange("d (p b) -> d p b", p=gp)
                x_bc = x_t[k][:].unsqueeze(1).broadcast_to([KP, gp, B])
                w_bc = (wm_t[k][:, p_start:p_start + gp]
                        .unsqueeze(2).broadcast_to([KP, gp, B]))
                nc.vector.tensor_mul(st3, x_bc, w_bc)
                for pair in range(npair):
                    lhsT = st[:, 2 * B * pair:2 * B * (pair + 1)]
                    for n in range(NCH):
                        nc.tensor.matmul(
                            ps[NCH * pair + n][:],
                            lhsT,
                            g_r[k][:, NC_N * n:NC_N * (n + 1)],
                            start=(k == 0),
                            stop=(k == KT - 1),
                        )
            for pair in range(npair):
                pg = p_start // 2 + pair       # global pair index 0..15
                last = (g == len(group_sizes) - 1 and pair == npair - 1)
                o = outs.tile([2 * B, E], F32, tag="o", name=f"o{g}_{pair}")
                p0 = 2 * pg
                for n in range(NCH):
                    nc.scalar.activation(
                        o[:, NC_N * n:NC_N * (n + 1)], ps[NCH * pair + n][:],
                        mybir.ActivationFunctionType.Identity,
                        bias=bias_t[:, pg:pg + 1],
                    )
                    if last:
                        dst = out_p[p0:p0 + 2, :, NC_N * n:NC_N * (n + 1)]
                        ring(odma[0]).dma_start(
                            out=dst, in_=o[:, NC_N * n:NC_N * (n + 1)])
                        odma[0] += 1
                if not last:
                    dst = out_p[p0:p0 + 2, :, :]
                    ring(odma[0]).dma_start(out=dst, in_=o[:])
                    odma[0] += 1
            p_start += gp

        if repeat > 1:
            loop_cm.__exit__(None, None, None)

    nc.finalize()
    return nc


_NC_CACHE = None


def _get_program():
    global _NC_CACHE
    if _NC_CACHE is None:
        _NC_CACHE = _build_program()
    return _NC_CACHE


def _make_in_maps(x, weight, bias, mask, gene_embedding):
    def kperm(a):  # (D, W) -> (KP, KT*W) with [d, k*W+w] = a[k*KP+d, w]
        w = a.shape[1]
        return np.ascontiguousarray(
            a.reshape(KT, KP, w).transpose(1, 0, 2).reshape(KP, KT * w))

    xT = x.T.reshape(KT, KP, B)                          # (16, 125, 64)
    in_maps = []
    for c in range(N_CORES):
        sl = slice(P_CORE * c, P_CORE * (c + 1))
        wT_c = weight[sl].T.reshape(KT, KP, P_CORE)
        mk_c = mask[:, sl].reshape(KT, KP, P_CORE)
        # k-major blocks [x_k | w_k | m_k] -> (125, 16*(64+32+32))
        xwm = np.ascontiguousarray(
            np.concatenate([xT, wT_c, mk_c], axis=2)
            .transpose(1, 0, 2).reshape(KP, -1))
        b_c = bias[sl]
        # (128, 16): col i = [bias[2i]]*64 ++ [bias[2i+1]]*64
        bias_sb = np.ascontiguousarray(
            np.repeat(b_c.reshape(P_CORE // 2, 2), B, axis=1).T)
        in_maps.append({"xwm": xwm, "g": gene_embedding, "bias": bias_sb})
    return in_maps


def kernel(x, weight, bias, mask, gene_embedding, _want_results=False, **_):
    x = np.ascontiguousarray(x, dtype=np.float32)
    weight = np.ascontiguousarray(weight, dtype=np.float32)
    bias = np.ascontiguousarray(bias, dtype=np.float32)
    mask = np.ascontiguousarray(mask, dtype=np.float32)
    g = np.ascontiguousarray(gene_embedding, dtype=np.float32)

    in_maps = _make_in_maps(x, weight, bias, mask, g)
    nc = _get_program()
    res = run_bass_kernel_spmd(nc, in_maps, list(range(N_CORES)))
    out = np.concatenate([r["out"] for r in res.results], axis=1)
    if _want_results:
        return out, res
    return out


# revision 24
# speedup vs baseline: 1.1122x; 1.0140x over previous
"""Trainium2 Bass kernel for nn_CustomizedLinear (masked pathway linear).

out[b, p*768+e] = sum_d x[b,d] * (weight*mask.T)[p,d] * G[d,e] + bias[p]
with B=64, P=256, D=2000, E=768.

Sharding: tensor-parallel over the pathway dim P — 32 pathways per core on
8 cores; x and gene_embedding replicated.

Per-core compute: for each pathway p, scale x columns by wm[p] (DVE
broadcast multiply) and matmul with G. The TensorE matmul costs N cycles
per instruction regardless of K/M, so we pack 2 pathways x 64 batch rows
into the stationary operand (M=128) and stream G in N=384 chunks,
accumulating 16 k-tiles of 125 into PSUM. All matmul operands are
float32r (fp32 with 11-bit mantissa, 1 cycle/row vs 4 for fp32; rel err
~1.5e-4 at this depth). Input/output DMAs are split across both HWDGE
rings (SP + Activation) so G streaming does not starve the PE.
"""
import sys

sys.path.insert(0, "/opt/trn_rl_repo")

import numpy as np
from contextlib import ExitStack

import concourse.bacc as bacc
import concourse.tile as tile
import concourse.mybir as mybir
from concourse.bass_utils import run_bass_kernel_spmd

F32 = mybir.dt.float32
F32R = mybir.dt.float32r

N_CORES = 8
B = 64          # batch
D = 2000        # genes (contraction)
E = 768         # embedding
P_TOT = 256     # pathways
P_CORE = P_TOT // N_CORES        # 32 pathways per core
KT = 16                          # k-tiles
KP = D // KT                     # 125 rows per k-tile
NCH = 2                          # N chunks per pair
NC_N = E // NCH                  # 384


def _build_program(repeat=1, group_sizes=(8, 4, 4, 4, 4, 4, 2, 2),
                   split_rings=True, psum_bufs=8,
                   strip_bufs=6, g_chunks=(1,) * KT, g_rings=(0, 0, 1)):
    assert sum(group_sizes) == P_CORE
    nc = bacc.Bacc()
    # x/w/m arrive host-permuted k-major: per k-tile a contiguous block
    # [x_k (B) | w_k (P_CORE) | m_k (P_CORE)]; a small head DMA (k=0,1)
    # lets the strip pipeline start before the bulk load finishes
    BLK = B + 2 * P_CORE
    XWM_W = KT * BLK
    HEAD = 4
    xwm_d = nc.declare_dram_parameter("xwm", [KP, XWM_W], F32, isOutput=False)
    g_d = nc.declare_dram_parameter("g", [D, E], F32, isOutput=False)
    bias_d = nc.declare_dram_parameter("bias", [2 * B, P_CORE // 2], F32,
                                       isOutput=False)
    out_d = nc.declare_dram_parameter("out", [B, P_CORE * E], F32, isOutput=True)

    def ring(i):
        if not split_rings:
            return nc.sync
        return nc.sync if i % 2 == 0 else nc.scalar

    with tile.TileContext(nc) as tc, ExitStack() as ctx:
        const = ctx.enter_context(tc.tile_pool(name="const", bufs=1))
        stage = ctx.enter_context(tc.tile_pool(name="stage", bufs=3))
        strips = ctx.enter_context(tc.tile_pool(name="strips", bufs=strip_bufs))
        outs = ctx.enter_context(tc.tile_pool(name="outs", bufs=4))
        psum = ctx.enter_context(
            tc.tile_pool(name="psum", bufs=psum_bufs, space="PSUM"))

        # x/w/m: head (k<HEAD) now; tail spliced into the G stream below
        xwm_h = const.tile([KP, HEAD * BLK], F32)
        nc.scalar.dma_start(out=xwm_h[:], in_=xwm_d[:, :HEAD * BLK])
        bias_t = const.tile([2 * B, P_CORE // 2], F32)
        nc.scalar.dma_start(out=bias_t[:], in_=bias_d[:])
        MID = 10
        xwm_t1 = const.tile([KP, (MID - HEAD) * BLK], F32)
        xwm_t2 = const.tile([KP, (KT - MID) * BLK], F32)

        def blk(k):
            if k < HEAD:
                return xwm_h[:, BLK * k:BLK * (k + 1)]
            if k < MID:
                o = BLK * (k - HEAD)
                return xwm_t1[:, o:o + BLK]
            o = BLK * (k - MID)
            return xwm_t2[:, o:o + BLK]

        x_t, wm_t = [None] * KT, [None] * KT

        def emit_wm(ka, kb):
            for k in range(ka, kb):
                b = blk(k)
                x_t[k] = b[:, :B]
                wm = const.tile([KP, P_CORE], F32, tag=f"wm{k}",
                                name=f"wm{k}")
                nc.vector.tensor_mul(wm[:], b[:, B:B + P_CORE],
                                     b[:, B + P_CORE:])
                wm_t[k] = wm

        emit_wm(0, HEAD)

        # G stream: uneven chunks so the first cast starts early; ring
        # placement per g_rings; casts to f32r on the idle gpsimd engine
        g_view = g_d[:].rearrange("(k d) e -> d k e", k=KT)
        g_r = []
        k0 = 0
        for c, w in enumerate(g_chunks):
            if c == 2:  # xwm tail pt1 after G has a head start
                nc.scalar.dma_start(out=xwm_t1[:],
                                    in_=xwm_d[:, HEAD * BLK:MID * BLK])
                emit_wm(HEAD, MID)
            if c == 4:
                nc.scalar.dma_start(out=xwm_t2[:], in_=xwm_d[:, MID * BLK:])
                emit_wm(MID, KT)
            gs = const.tile([KP, w * E], F32, name=f"gs{c}")
            dst = gs[:].rearrange("d (k e) -> d k e", k=w)
            eng = nc.sync if g_rings[c % len(g_rings)] == 0 else nc.scalar
            eng.dma_start(out=dst, in_=g_view[:, k0:k0 + w, :])
            for j in range(w):
                gr = const.tile([KP, E], F32R, tag=f"g{k0 + j}",
                                name=f"g{k0 + j}")
                cast_eng = nc.vector if k0 + j < 1 else nc.gpsimd
                cast_eng.tensor_copy(gr[:], gs[:, E * j:E * (j + 1)])
                g_r.append(gr)
            k0 += w
        assert k0 == KT


        out_p = out_d[:].rearrange("b (p e) -> p b e", p=P_CORE)  # [32, 64, 768]

        if repeat > 1:
            loop_cm = tc.For_i(0, repeat, 1,
                               hint_engines=(mybir.EngineType.PE,))
            loop_cm.__enter__()

        odma = [0]
        p_start = 0
        for g, gp in enumerate(group_sizes):
            npair = gp // 2
            ps = [psum.tile([2 * B, NC_N], F32, tag="ps", name=f"ps{g}_{i}")
                  for i in range(npair * NCH)]
            for k in range(KT):
                st = strips.tile([KP, gp * B], F32R, tag=f"strip{gp}",
                                 name=f"st{g}_{k}")
                st3 = st[:].rearrange("d (p b) -> d p b", p=gp)
                x_bc = x_t[k][:].unsqueeze(1).broadcast_to([KP, gp, B])
                w_bc = (wm_t[k][:, p_start:p_start + gp]
                        .unsqueeze(2).broadcast_to([KP, gp, B]))
                nc.vector.tensor_mul(st3, x_bc, w_bc)
                for pair in range(npair):
                    lhsT = st[:, 2 * B * pair:2 * B * (pair + 1)]
                    for n in range(NCH):
                        nc.tensor.matmul(
                            ps[NCH * pair + n][:],
                            lhsT,
                            g_r[k][:, NC_N * n:NC_N * (n + 1)],
                            start=(k == 0),
                            stop=(k == KT - 1),
                        )
            for pair in range(npair):
                pg = p_start // 2 + pair       # global pair index 0..15
                last = (g == len(group_sizes) - 1 and pair == npair - 1)
                o = outs.tile([2 * B, E], F32, tag="o", name=f"o{g}_{pair}")
                p0 = 2 * pg
                for n in range(NCH):
                    nc.scalar.activation(
                        o[:, NC_N * n:NC_N * (n + 1)], ps[NCH * pair + n][:],
                        mybir.ActivationFunctionType.Identity,
                        bias=bias_t[:, pg:pg + 1],
                    )
                    if last:
                        dst = out_p[p0:p0 + 2, :, NC_N * n:NC_N * (n + 1)]
                        ring(odma[0]).dma_start(
                            out=dst, in_=o[:, NC_N * n:NC_N * (n + 1)])
                        odma[0] += 1
                if not last:
                    dst = out_p[p0:p0 + 2, :, :]
                    ring(odma[0]).dma_start(out=dst, in_=o[:])
                    odma[0] += 1
            p_start += gp

        if repeat > 1:
            loop_cm.__exit__(None, None, None)

    nc.finalize()
    return nc


_NC_CACHE = None


def _get_program():
    global _NC_CACHE
    if _NC_CACHE is None:
        _NC_CACHE = _build_program()
    return _NC_CACHE


def _make_in_maps(x, weight, bias, mask, gene_embedding):
    def kperm(a):  # (D, W) -> (KP, KT*W) with [d, k*W+w] = a[k*KP+d, w]
        w = a.shape[1]
        return np.ascontiguousarray(
            a.reshape(KT, KP, w).transpose(1, 0, 2).reshape(KP, KT * w))

    xT = x.T.reshape(KT, KP, B)                          # (16, 125, 64)
    in_maps = []
    for c in range(N_CORES):
        sl = slice(P_CORE * c, P_CORE * (c + 1))
        wT_c = weight[sl].T.reshape(KT, KP, P_CORE)
        mk_c = mask[:, sl].reshape(KT, KP, P_CORE)
        # k-major blocks [x_k | w_k | m_k] -> (125, 16*(64+32+32))
        xwm = np.ascontiguousarray(
            np.concatenate([xT, wT_c, mk_c], axis=2)
            .transpose(1, 0, 2).reshape(KP, -1))
        b_c = bias[sl]
        # (128, 16): col i = [bias[2i]]*64 ++ [bias[2i+1]]*64
        bias_sb = np.ascontiguousarray(
            np.repeat(b_c.reshape(P_CORE // 2, 2), B, axis=1).T)
        in_maps.append({"xwm": xwm, "g": gene_embedding, "bias": bias_sb})
    return in_maps


def kernel(x, weight, bias, mask, gene_embedding, _want_results=False, **_):
    x = np.ascontiguousarray(x, dtype=np.float32)
    weight = np.ascontiguousarray(weight, dtype=np.float32)
    bias = np.ascontiguousarray(bias, dtype=np.float32)
    mask = np.ascontiguousarray(mask, dtype=np.float32)
    g = np.ascontiguousarray(gene_embedding, dtype=np.float32)

    in_maps = _make_in_maps(x, weight, bias, mask, g)
    nc = _get_program()
    res = run_bass_kernel_spmd(nc, in_maps, list(range(N_CORES)))
    out = np.concatenate([r["out"] for r in res.results], axis=1)
    if _want_results:
        return out, res
    return out


# revision 25
# speedup vs baseline: 1.1259x; 1.0124x over previous
"""Trainium2 Bass kernel for nn_CustomizedLinear (masked pathway linear).

out[b, p*768+e] = sum_d x[b,d] * (weight*mask.T)[p,d] * G[d,e] + bias[p]
with B=64, P=256, D=2000, E=768.

Sharding: tensor-parallel over the pathway dim P — 32 pathways per core on
8 cores; x and gene_embedding replicated.

Per-core compute: for each pathway p, scale x columns by wm[p] (DVE
broadcast multiply) and matmul with G. The TensorE matmul costs N cycles
per instruction regardless of K/M, so we pack 2 pathways x 64 batch rows
into the stationary operand (M=128) and stream G in N=384 chunks,
accumulating 16 k-tiles of 125 into PSUM. All matmul operands are
float32r (fp32 with 11-bit mantissa, 1 cycle/row vs 4 for fp32; rel err
~1.5e-4 at this depth). Input/output DMAs are split across both HWDGE
rings (SP + Activation) so G streaming does not starve the PE.
"""
import sys

sys.path.insert(0, "/opt/trn_rl_repo")

import numpy as np
from contextlib import ExitStack

import concourse.bacc as bacc
import concourse.tile as tile
import concourse.mybir as mybir
from concourse.bass_utils import run_bass_kernel_spmd

F32 = mybir.dt.float32
F32R = mybir.dt.float32r

N_CORES = 8
B = 64          # batch
D = 2000        # genes (contraction)
E = 768         # embedding
P_TOT = 256     # pathways
P_CORE = P_TOT // N_CORES        # 32 pathways per core
KT = 16                          # k-tiles
KP = D // KT                     # 125 rows per k-tile
NCH = 2                          # N chunks per pair
NC_N = E // NCH                  # 384


def _build_program(repeat=1, group_sizes=(8, 4, 4, 4, 4, 4, 2, 2),
                   split_rings=True, psum_bufs=8,
                   strip_bufs=6, g_chunks=(1,) * KT, g_rings=(0, 0, 1)):
    assert sum(group_sizes) == P_CORE
    nc = bacc.Bacc()
    # x/w/m arrive host-permuted k-major: per k-tile a contiguous block
    # [x_k (B) | w_k (P_CORE) | m_k (P_CORE)]; a small head DMA (k=0,1)
    # lets the strip pipeline start before the bulk load finishes
    BLK = B + 2 * P_CORE
    XWM_W = KT * BLK
    HEAD = 2
    xwm_d = nc.declare_dram_parameter("xwm", [KP, XWM_W], F32, isOutput=False)
    g_d = nc.declare_dram_parameter("g", [D, E], F32, isOutput=False)
    bias_d = nc.declare_dram_parameter("bias", [2 * B, P_CORE // 2], F32,
                                       isOutput=False)
    out_d = nc.declare_dram_parameter("out", [B, P_CORE * E], F32, isOutput=True)

    def ring(i):
        if not split_rings:
            return nc.sync
        return nc.sync if i % 2 == 0 else nc.scalar

    with tile.TileContext(nc) as tc, ExitStack() as ctx:
        const = ctx.enter_context(tc.tile_pool(name="const", bufs=1))
        stage = ctx.enter_context(tc.tile_pool(name="stage", bufs=3))
        strips = ctx.enter_context(tc.tile_pool(name="strips", bufs=strip_bufs))
        outs = ctx.enter_context(tc.tile_pool(name="outs", bufs=4))
        psum = ctx.enter_context(
            tc.tile_pool(name="psum", bufs=psum_bufs, space="PSUM"))

        # x/w/m: head (k<HEAD) now; tail spliced into the G stream below
        xwm_h = const.tile([KP, HEAD * BLK], F32)
        nc.scalar.dma_start(out=xwm_h[:], in_=xwm_d[:, :HEAD * BLK])
        bias_t = const.tile([2 * B, P_CORE // 2], F32)
        nc.scalar.dma_start(out=bias_t[:], in_=bias_d[:])
        MID = 10
        xwm_t1 = const.tile([KP, (MID - HEAD) * BLK], F32)
        xwm_t2 = const.tile([KP, (KT - MID) * BLK], F32)

        def blk(k):
            if k < HEAD:
                return xwm_h[:, BLK * k:BLK * (k + 1)]
            if k < MID:
                o = BLK * (k - HEAD)
                return xwm_t1[:, o:o + BLK]
            o = BLK * (k - MID)
            return xwm_t2[:, o:o + BLK]

        x_t, wm_t = [None] * KT, [None] * KT

        def emit_wm(ka, kb):
            for k in range(ka, kb):
                b = blk(k)
                x_t[k] = b[:, :B]
                wm = const.tile([KP, P_CORE], F32, tag=f"wm{k}",
                                name=f"wm{k}")
                nc.vector.tensor_mul(wm[:], b[:, B:B + P_CORE],
                                     b[:, B + P_CORE:])
                wm_t[k] = wm

        emit_wm(0, HEAD)

        # G stream: uneven chunks so the first cast starts early; ring
        # placement per g_rings; casts to f32r on the idle gpsimd engine
        g_view = g_d[:].rearrange("(k d) e -> d k e", k=KT)
        g_r = []
        k0 = 0
        for c, w in enumerate(g_chunks):
            if c == 2:  # xwm tail pt1 after G has a head start
                nc.scalar.dma_start(out=xwm_t1[:],
                                    in_=xwm_d[:, HEAD * BLK:MID * BLK])
                emit_wm(HEAD, MID)
            if c == 4:
                nc.scalar.dma_start(out=xwm_t2[:], in_=xwm_d[:, MID * BLK:])
                emit_wm(MID, KT)
            gs = const.tile([KP, w * E], F32, name=f"gs{c}")
            dst = gs[:].rearrange("d (k e) -> d k e", k=w)
            eng = nc.sync if g_rings[c % len(g_rings)] == 0 else nc.scalar
            eng.dma_start(out=dst, in_=g_view[:, k0:k0 + w, :])
            for j in range(w):
                gr = const.tile([KP, E], F32R, tag=f"g{k0 + j}",
                                name=f"g{k0 + j}")
                cast_eng = nc.vector if k0 + j < 1 else nc.gpsimd
                cast_eng.tensor_copy(gr[:], gs[:, E * j:E * (j + 1)])
                g_r.append(gr)
            k0 += w
        assert k0 == KT


        out_p = out_d[:].rearrange("b (p e) -> p b e", p=P_CORE)  # [32, 64, 768]

        if repeat > 1:
            loop_cm = tc.For_i(0, repeat, 1,
                               hint_engines=(mybir.EngineType.PE,))
            loop_cm.__enter__()

        odma = [0]
        p_start = 0
        for g, gp in enumerate(group_sizes):
            npair = gp // 2
            ps = [psum.tile([2 * B, NC_N], F32, tag="ps", name=f"ps{g}_{i}")
                  for i in range(npair * NCH)]
            for k in range(KT):
                st = strips.tile([KP, gp * B], F32R, tag=f"strip{gp}",
                                 name=f"st{g}_{k}")
                st3 = st[:].rearrange("d (p b) -> d p b", p=gp)
                x_bc = x_t[k][:].unsqueeze(1).broadcast_to([KP, gp, B])
                w_bc = (wm_t[k][:, p_start:p_start + gp]
                        .unsqueeze(2).broadcast_to([KP, gp, B]))
                nc.vector.tensor_mul(st3, x_bc, w_bc)
                for pair in range(npair):
                    lhsT = st[:, 2 * B * pair:2 * B * (pair + 1)]
                    for n in range(NCH):
                        nc.tensor.matmul(
                            ps[NCH * pair + n][:],
                            lhsT,
                            g_r[k][:, NC_N * n:NC_N * (n + 1)],
                            start=(k == 0),
                            stop=(k == KT - 1),
                        )
            for pair in range(npair):
                pg = p_start // 2 + pair       # global pair index 0..15
                last = (g == len(group_sizes) - 1 and pair == npair - 1)
                o = outs.tile([2 * B, E], F32, tag="o", name=f"o{g}_{pair}")
                p0 = 2 * pg
                for n in range(NCH):
                    nc.scalar.activation(
                        o[:, NC_N * n:NC_N * (n + 1)], ps[NCH * pair + n][:],
                        mybir.ActivationFunctionType.Identity,
                        bias=bias_t[:, pg:pg + 1],
                    )
                    if last:
                        dst = out_p[p0:p0 + 2, :, NC_N * n:NC_N * (n + 1)]
                        ring(odma[0]).dma_start(
                            out=dst, in_=o[:, NC_N * n:NC_N * (n + 1)])
                        odma[0] += 1
                if not last:
                    dst = out_p[p0:p0 + 2, :, :]
                    ring(odma[0]).dma_start(out=dst, in_=o[:])
                    odma[0] += 1
            p_start += gp

        if repeat > 1:
            loop_cm.__exit__(None, None, None)

    nc.finalize()
    return nc


_NC_CACHE = None


def _get_program():
    global _NC_CACHE
    if _NC_CACHE is None:
        _NC_CACHE = _build_program()
    return _NC_CACHE


def _make_in_maps(x, weight, bias, mask, gene_embedding):
    def kperm(a):  # (D, W) -> (KP, KT*W) with [d, k*W+w] = a[k*KP+d, w]
        w = a.shape[1]
        return np.ascontiguousarray(
            a.reshape(KT, KP, w).transpose(1, 0, 2).reshape(KP, KT * w))

    xT = x.T.reshape(KT, KP, B)                          # (16, 125, 64)
    in_maps = []
    for c in range(N_CORES):
        sl = slice(P_CORE * c, P_CORE * (c + 1))
        wT_c = weight[sl].T.reshape(KT, KP, P_CORE)
        mk_c = mask[:, sl].reshape(KT, KP, P_CORE)
        # k-major blocks [x_k | w_k | m_k] -> (125, 16*(64+32+32))
        xwm = np.ascontiguousarray(
            np.concatenate([xT, wT_c, mk_c], axis=2)
            .transpose(1, 0, 2).reshape(KP, -1))
        b_c = bias[sl]
        # (128, 16): col i = [bias[2i]]*64 ++ [bias[2i+1]]*64
        bias_sb = np.ascontiguousarray(
            np.repeat(b_c.reshape(P_CORE // 2, 2), B, axis=1).T)
        in_maps.append({"xwm": xwm, "g": gene_embedding, "bias": bias_sb})
    return in_maps


def kernel(x, weight, bias, mask, gene_embedding, _want_results=False, **_):
    x = np.ascontiguousarray(x, dtype=np.float32)
    weight = np.ascontiguousarray(weight, dtype=np.float32)
    bias = np.ascontiguousarray(bias, dtype=np.float32)
    mask = np.ascontiguousarray(mask, dtype=np.float32)
    g = np.ascontiguousarray(gene_embedding, dtype=np.float32)

    in_maps = _make_in_maps(x, weight, bias, mask, g)
    nc = _get_program()
    res = run_bass_kernel_spmd(nc, in_maps, list(range(N_CORES)))
    out = np.concatenate([r["out"] for r in res.results], axis=1)
    if _want_results:
        return out, res
    return out
